# revision 27
# baseline (speedup 1.0000x reference)
"""Trainium2 Bass kernel for the Affine grid-sample problem.

reference: F.affine_grid(theta, align_corners=True) + F.grid_sample(x,
bilinear, zeros, align_corners=True) with x [8, 16, 512, 512] f32 and a
batch-broadcast theta [2, 3].

For the (diagonal) theta used by the problem the sampling grid is
separable: ix depends only on the output column j, iy only on the output
row i.  Bilinear sampling then factors into two banded matrices built on
the host:

    out[b, c] = Ry @ x[b, c] @ Cx^T        (Ry [H,H], Cx [W,W], 2 nnz/row)

On the device each 512x512 image is processed as two TensorE stages with
the *image tile* as the stationary operand (so no transposes are needed):

  stage A:  tT[w, i] = sum_y x[y, w] * RyT[y, i]      (psum: [w-block, i])
  stage B: out[i, j] = sum_w tT[w, i] * CxT[w, j]     (psum: [i-block, j])

The interp matrices are banded, so for each 128-row contraction tile only
a contiguous chunk of output columns is nonzero; we only stream those
columns (host-packed operands), cutting TensorE work ~4x vs dense.

Sharding: pure data parallel over the batch (8 cores x 1 image of
[16, 512, 512]); the packed interp matrices are replicated to all cores.
No collectives are needed (forward only).
"""

import numpy as np

import concourse.bass as bass
import concourse.bacc as bacc
import concourse.mybir as mybir
import concourse.tile as tile
from concourse.bass_utils import run_bass_kernel_spmd

B, C, H, W = 8, 16, 512, 512
P = 128
N_CORES = 8
QH = H // P  # 4 row tiles
QW = W // P  # 4 col tiles

# compute dtype for the matmuls: "float32" | "float16" | "bfloat16" | "float32r"
# float16: ~104 us HW exec, absmax err ~4e-3 = 8.2e-4 of |ref|_max
# float32: ~241 us HW exec, absmax err ~7e-7 (fp32-exact, 4x slower:
#          fp32 matmuls run as two half-rate passes with 4-byte LDWEIGHTS)
# the e2e is DMA-bound either way at ~32 MB HBM traffic/core (~89 us floor)
COMPUTE = "float16"

_f32 = np.float32


# ----------------------------------------------------------------- host math
def _grids(theta):
    """Mirror the reference's fp32 grid math. Returns (ix[H,W], iy[H,W])."""
    theta = np.asarray(theta, dtype=_f32)
    try:
        import jax
        import jax.numpy as jnp

        cpu = jax.devices("cpu")[0]
        with jax.default_device(cpu):
            xs = jnp.linspace(-1.0, 1.0, W, dtype=jnp.float32)
            ys = jnp.linspace(-1.0, 1.0, H, dtype=jnp.float32)
            X, Y = jnp.meshgrid(xs, ys)
            gx = theta[0, 0] * X + theta[0, 1] * Y + theta[0, 2]
            gy = theta[1, 0] * X + theta[1, 1] * Y + theta[1, 2]
            ix = (gx + 1.0) * 0.5 * (W - 1)
            iy = (gy + 1.0) * 0.5 * (H - 1)
            return np.asarray(ix), np.asarray(iy)
    except Exception:
        xs = np.linspace(-1.0, 1.0, W).astype(_f32)
        ys = np.linspace(-1.0, 1.0, H).astype(_f32)
        X, Y = np.meshgrid(xs, ys)
        gx = (theta[0, 0] * X + theta[0, 1] * Y + theta[0, 2]).astype(_f32)
        gy = (theta[1, 0] * X + theta[1, 1] * Y + theta[1, 2]).astype(_f32)
        ix = ((gx + _f32(1.0)) * _f32(0.5) * _f32(W - 1)).astype(_f32)
        iy = ((gy + _f32(1.0)) * _f32(0.5) * _f32(H - 1)).astype(_f32)
        return ix, iy


def _corners(coord):
    """coord [N] fp32 -> per-column list of valid (src_idx, weight)."""
    size = len(coord)
    i0 = np.floor(coord)
    frac = (coord - i0).astype(_f32)
    i0 = i0.astype(np.int64)
    out = []
    for c in range(size):
        lst = []
        if 0 <= i0[c] < size:
            lst.append((int(i0[c]), _f32(1.0) - frac[c]))
        if 0 <= i0[c] + 1 < size:
            lst.append((int(i0[c]) + 1, frac[c]))
        out.append(lst)
    return out


def _chunk_plan(corners, size, bases=None):
    """Partition output columns into per-contraction-tile matmul ops.

    Returns (ops, packed): ops is a list of (tile_k, lo, hi_inclusive,
    packed_col_offset).  Main ops have pairwise-disjoint [lo, hi] ranges
    covering [0, size); "seam" ops (1-2 columns at tile boundaries, where
    a column's two source rows straddle two contraction tiles) re-touch
    columns already written by an earlier op and accumulate.  Each op's
    range is uniformly fresh or uniformly re-touched, which both the HW
    per-element has_written rule and the sim's region rule accept.
    packed is the [P, total] fp32 moving operand: weight of source row
    tile_k*P+p for output column lo+j at packed[p, off+j]."""
    iv = {}
    for c in range(size):
        for idx, _w in corners[c]:
            k = idx // P
            lo, hi = iv.get(k, (c, c))
            iv[k] = (min(lo, c), max(hi, c))
    assert iv, "no valid sample points at all"
    ks = sorted(iv)
    # all-out-of-bounds columns are pure zeros; covering them with
    # zero-weight matmul columns is cheaper than fragmenting the stores
    # (a trimmed-store variant measured slower end-to-end).
    first, last = ks[0], ks[-1]
    iv[first] = (0, iv[first][1])
    iv[last] = (iv[last][0], size - 1)
    cmin, cmax = 0, size - 1
    for a, b in zip(ks, ks[1:]):
        if iv[a][1] + 1 < iv[b][0]:  # gap: columns with no valid corners
            iv[a] = (iv[a][0], iv[b][0] - 1)
    covered = np.zeros(size, dtype=bool)
    ops = []
    prev_hi = -1
    for k in ks:
        lo, hi = iv[k]
        fresh_lo = max(lo, prev_hi + 1)
        if lo <= min(prev_hi, hi):  # seam columns, accumulate
            ops.append((k, lo, min(prev_hi, hi)))
        if fresh_lo <= hi:  # fresh columns, overwrite
            ops.append((k, fresh_lo, hi))
            covered[fresh_lo : hi + 1] = True
        prev_hi = max(prev_hi, hi)
    assert covered.all(), "chunk plan does not cover all output columns"
    out_ops = []
    off = 0
    for k, lo, hi in ops:
        out_ops.append((k, lo, hi, off))
        off += hi - lo + 1
    packed = np.zeros((P, off), dtype=_f32)
    for k, lo, hi, o in out_ops:
        base = bases[k] if bases is not None else P * k
        for c in range(lo, hi + 1):
            for idx, wt in corners[c]:
                if idx // P == k:
                    packed[idx - base, o + c - lo] += wt
    return out_ops, packed, cmin, cmax


def _np_fallback(x, ix, iy):
    """Direct numpy implementation (general theta)."""
    x0 = np.floor(ix)
    y0 = np.floor(iy)
    wx = (ix - x0).astype(_f32)
    wy = (iy - y0).astype(_f32)
    x0i = x0.astype(np.int64)
    y0i = y0.astype(np.int64)
    out = np.zeros(x.shape, dtype=_f32)
    for dy in (0, 1):
        for dx in (0, 1):
            yi = y0i + dy
            xi = x0i + dx
            valid = ((xi >= 0) & (xi < W) & (yi >= 0) & (yi < H)).astype(_f32)
            yc = np.clip(yi, 0, H - 1)
            xc = np.clip(xi, 0, W - 1)
            wgt = (wy if dy else 1.0 - wy) * (wx if dx else 1.0 - wx) * valid
            out += x[:, :, yc, xc] * wgt.astype(_f32)
    return out.astype(_f32)


# ------------------------------------------------------------- bass program
def _np_dt(compute):
    if compute == "bfloat16":
        import ml_dtypes

        return np.dtype(ml_dtypes.bfloat16)
    if compute == "float16":
        return np.dtype(np.float16)
    return np.float32


def _bir_dt(compute):
    return {
        "float32": mybir.dt.float32,
        "float32r": mybir.dt.float32r,
        "bfloat16": mybir.dt.bfloat16,
        "float16": mybir.dt.float16,
    }[compute]


def _build_program(
    chunks_a, len_a, chunks_b, len_b, w_lo, w_hi, i_lo, i_hi, j_lo, j_hi, compute
):
    cdt = _bir_dt(compute)
    f32 = mybir.dt.float32
    cast_in = compute in ("bfloat16", "float16")
    WW = w_hi - w_lo + 1  # loaded x-column window (source cols ever sampled)
    # per w-tile m: loaded sub-window [wb[m], wb[m]+wn[m])
    wb = [max(P * m, w_lo) for m in range(QW)]
    wn = [max(0, min(P * m + P, w_hi + 1) - wb[m]) for m in range(QW)]
    JW = j_hi - j_lo + 1  # nonzero output column window

    nc = bacc.Bacc()
    x_in = nc.dram_tensor("x", [C, H, W], f32, kind="ExternalInput")
    ryt_in = nc.dram_tensor("ryt", [P, len_a], cdt, kind="ExternalInput")
    cxt_in = nc.dram_tensor("cxt", [P, len_b], cdt, kind="ExternalInput")
    out_ext = nc.dram_tensor("out", [C, H, W], f32, kind="ExternalOutput")

    with tile.TileContext(nc) as tc:
        with (
            tc.tile_pool(name="consts", bufs=1) as consts,
            tc.tile_pool(name="xp", bufs=3 * QH) as xp,
            tc.tile_pool(name="tp", bufs=2) as tp,
            tc.tile_pool(name="op", bufs=6) as op,
            tc.tile_pool(name="psa", bufs=4, space="PSUM") as psa,
            tc.tile_pool(name="psb", bufs=4, space="PSUM") as psb,
        ):
            ryt_sb = consts.tile([P, len_a], cdt, tag="ryt")
            cxt_sb = consts.tile([P, len_b], cdt, tag="cxt")
            nc.sync.dma_start(out=ryt_sb[:], in_=ryt_in[:])
            nc.sync.dma_start(out=cxt_sb[:], in_=cxt_in[:])

            for c in range(C):
                # split the load per y-tile so stage A starts on tile 0
                # while tiles 1-3 are still in flight
                xs = []
                for k in range(QH):
                    x_k = xp.tile([P, WW], cdt, tag="x")
                    src = x_in[c][P * k : P * k + P, w_lo : w_hi + 1]
                    if cast_in:
                        nc.gpsimd.dma_start(out=x_k[:], in_=src)
                    else:
                        nc.sync.dma_start(out=x_k[:], in_=src)
                    xs.append(x_k)

                # stage A: tT[w, i] = sum_y x[y, w] * RyT[y, i]
                tT_sb = tp.tile([P, QW * H], cdt, tag="t")
                for m in range(QW):
                    if wn[m] == 0:
                        continue
                    ps = psa.tile([P, H], f32, tag="psa")
                    nmm = len(chunks_a)
                    xoff = wb[m] - w_lo
                    for ci, (k, lo, hi, off) in enumerate(chunks_a):
                        nc.tensor.matmul(
                            out=ps[: wn[m], lo : hi + 1],
                            lhsT=xs[k][:, xoff : xoff + wn[m]],
                            rhs=ryt_sb[:, off : off + hi - lo + 1],
                            start=(ci == 0),
                            stop=(ci == nmm - 1),
                        )
                    nc.vector.tensor_copy(
                        out=tT_sb[: wn[m], m * H + i_lo : m * H + i_hi + 1],
                        in_=ps[: wn[m], i_lo : i_hi + 1],
                    )

                # stage B: out[i, j] = sum_w tT[w, i] * CxT[w, j]
                # tT tile m holds partitions p = w - wb[m] for p < wn[m];
                # cxt packed rows are built with the same re-base.
                # split output per i-block so each store goes out as soon as
                # its psum is evacuated; all-zero exterior rows/cols are
                # never computed or stored (output buffers arrive zeroed)
                for mi in range(QH):
                    r0 = max(0, i_lo - P * mi)
                    r1 = min(P, i_hi + 1 - P * mi)
                    if r1 <= r0:
                        continue
                    nr = r1 - r0
                    ps = psb.tile([P, JW], f32, tag="psb")
                    nmm = len(chunks_b)
                    for ci, (k, lo, hi, off) in enumerate(chunks_b):
                        if wn[k] == 0:
                            continue
                        nc.tensor.matmul(
                            out=ps[:nr, lo - j_lo : hi - j_lo + 1],
                            lhsT=tT_sb[
                                : wn[k], k * H + mi * P + r0 : k * H + mi * P + r1
                            ],
                            rhs=cxt_sb[: wn[k], off : off + hi - lo + 1],
                            start=(ci == 0),
                            stop=(ci == nmm - 1),
                        )
                    out_mi = op.tile([P, JW], f32, tag="o")
                    nc.scalar.copy(out=out_mi[:nr], in_=ps[:nr, :])
                    st_eng = nc.sync if mi % 2 == 0 else nc.scalar
                    st_eng.dma_start(
                        out=out_ext[c][P * mi + r0 : P * mi + r1, j_lo : j_hi + 1],
                        in_=out_mi[:nr],
                    )

    nc.finalize()
    return nc


# ------------------------------------------------------------------- driver
def _make_runner(nc):
    """Cached mirror of bass2jax.run_bass_via_pjrt's multi-core path: build
    the jitted shard_map executable once and reuse it across kernel() calls
    (run_bass_kernel_spmd re-traces and re-jits on every invocation)."""
    import jax
    import concourse.mybir as _mybir
    from concourse import bass2jax
    from jax.experimental.shard_map import shard_map
    from jax.sharding import Mesh, PartitionSpec

    bass2jax.install_neuronx_cc_hook()
    assert nc.dbg_addr is None
    partition_name = nc.partition_id_tensor.name if nc.partition_id_tensor else None
    in_names, out_names, out_avals = [], [], []
    for alloc in nc.m.functions[0].allocations:
        if not isinstance(alloc, _mybir.MemoryLocationSet):
            continue
        name = alloc.memorylocations[0].name
        if alloc.kind == "ExternalInput":
            if name != partition_name:
                in_names.append(name)
        elif alloc.kind == "ExternalOutput":
            out_names.append(name)
            out_avals.append(
                jax.core.ShapedArray(
                    tuple(alloc.tensor_shape), _mybir.dt.np(alloc.dtype)
                )
            )
    n_params = len(in_names)
    all_in = list(in_names) + list(out_names)
    if partition_name is not None:
        all_in.append(partition_name)
    donate = tuple(range(n_params, n_params + len(out_names)))

    def _body(*args):
        operands = list(args)
        if partition_name is not None:
            operands.append(bass2jax.partition_id_tensor())
        return tuple(
            bass2jax._bass_exec_p.bind(
                *operands,
                out_avals=tuple(out_avals),
                in_names=tuple(all_in),
                out_names=tuple(out_names),
                lowering_input_output_aliases=(),
                sim_require_finite=True,
                sim_require_nnan=True,
                nc=nc,
            )
        )

    devices = jax.devices()[:N_CORES]
    mesh = Mesh(np.asarray(devices), ("core",))
    nio = n_params + len(out_names)
    sharded = jax.jit(
        shard_map(
            _body,
            mesh=mesh,
            in_specs=(PartitionSpec("core"),) * nio,
            out_specs=(PartitionSpec("core"),) * len(out_names),
            check_rep=False,
        ),
        donate_argnums=donate,
        keep_unused=True,
    )

    import jax.numpy as jnp
    from jax.sharding import NamedSharding

    # donated output seed buffers, created on-device (they are consumed by
    # donation every call; making them device-side avoids shipping ~134MB
    # of host zeros through the transport on each call)
    zero_shapes = [
        ((N_CORES * a.shape[0], *a.shape[1:]), a.dtype) for a in out_avals
    ]
    make_zeros = jax.jit(
        lambda: tuple(jnp.zeros(s, d) for s, d in zero_shapes),
        out_shardings=tuple(
            NamedSharding(mesh, PartitionSpec("core")) for _ in zero_shapes
        ),
    )

    def run(in_maps):
        concat_in = [
            np.concatenate([np.asarray(m[name]) for m in in_maps], axis=0)
            for name in in_names
        ]
        out_arrs = sharded(*concat_in, *make_zeros())
        return [
            {
                name: np.asarray(out_arrs[i]).reshape(N_CORES, *out_avals[i].shape)[c]
                for i, name in enumerate(out_names)
            }
            for c in range(N_CORES)
        ]

    return run


_cache = {}


def _prepare(theta, compute):
    key = (np.asarray(theta, dtype=_f32).tobytes(), compute)
    if key in _cache:
        return _cache[key]
    ix, iy = _grids(theta)
    sep = np.array_equal(ix, np.broadcast_to(ix[:1, :], ix.shape)) and np.array_equal(
        iy, np.broadcast_to(iy[:, :1], iy.shape)
    )
    if not sep:
        _cache[key] = (None, ix, iy)
        return _cache[key]
    corners_y = _corners(iy[:, 0])
    corners_x = _corners(ix[0, :])
    chunks_a, packed_a, i_lo, i_hi = _chunk_plan(corners_y, H)  # row stage
    all_x_idx = [idx for lst in corners_x for idx, _ in lst]
    w_lo, w_hi = min(all_x_idx), max(all_x_idx)
    wb = [max(P * m, w_lo) for m in range(QW)]
    chunks_b, packed_b, j_lo, j_hi = _chunk_plan(corners_x, W, bases=wb)  # col stage
    ndt = _np_dt(compute)
    nc = _build_program(
        chunks_a,
        packed_a.shape[1],
        chunks_b,
        packed_b.shape[1],
        w_lo,
        w_hi,
        i_lo,
        i_hi,
        j_lo,
        j_hi,
        compute,
    )
    state = (
        (nc, packed_a.astype(ndt), packed_b.astype(ndt)),
        ix,
        iy,
    )
    _cache[key] = state
    return state


_runners = {}


def _run(x, theta, trace=False, compute=None):
    compute = compute or COMPUTE
    x = np.ascontiguousarray(np.asarray(x, dtype=_f32))
    prog, ix, iy = _prepare(theta, compute)
    if prog is None:
        return _np_fallback(x, ix, iy), None
    nc, packed_a, packed_b = prog
    in_maps = [
        {"x": x[b], "ryt": packed_a, "cxt": packed_b} for b in range(N_CORES)
    ]
    res = None
    if trace:
        res = run_bass_kernel_spmd(nc, in_maps, list(range(N_CORES)), trace=True)
        results = res.results
    else:
        key = id(nc)
        try:
            if key not in _runners:
                _runners[key] = _make_runner(nc)
            results = _runners[key](in_maps)
        except Exception:
            res = run_bass_kernel_spmd(nc, in_maps, list(range(N_CORES)))
            results = res.results
    out = np.stack([np.asarray(results[b]["out"]) for b in range(N_CORES)])
    return out.astype(_f32), res


def _np_reference(x, theta):
    """Shape-generic numpy fallback (mirrors the reference directly)."""
    theta = np.asarray(theta, dtype=_f32)
    _, _, h, w = x.shape
    xs = np.linspace(-1.0, 1.0, w).astype(_f32)
    ys = np.linspace(-1.0, 1.0, h).astype(_f32)
    X, Y = np.meshgrid(xs, ys)
    gx = (theta[0, 0] * X + theta[0, 1] * Y + theta[0, 2]).astype(_f32)
    gy = (theta[1, 0] * X + theta[1, 1] * Y + theta[1, 2]).astype(_f32)
    ix = ((gx + _f32(1.0)) * _f32(0.5) * _f32(w - 1)).astype(_f32)
    iy = ((gy + _f32(1.0)) * _f32(0.5) * _f32(h - 1)).astype(_f32)
    x0 = np.floor(ix)
    y0 = np.floor(iy)
    wx = (ix - x0).astype(_f32)
    wy = (iy - y0).astype(_f32)
    x0i = x0.astype(np.int64)
    y0i = y0.astype(np.int64)
    out = np.zeros(x.shape, dtype=_f32)
    for dy in (0, 1):
        for dx in (0, 1):
            yi = y0i + dy
            xi = x0i + dx
            valid = ((xi >= 0) & (xi < w) & (yi >= 0) & (yi < h)).astype(_f32)
            yc = np.clip(yi, 0, h - 1)
            xc = np.clip(xi, 0, w - 1)
            wgt = (wy if dy else 1.0 - wy) * (wx if dx else 1.0 - wx) * valid
            out += x[:, :, yc, xc] * wgt.astype(_f32)
    return out.astype(_f32)


def kernel(x, theta):
    x = np.asarray(x)
    if x.shape != (B, C, H, W):
        return _np_reference(np.ascontiguousarray(x, dtype=_f32), theta)
    out, _ = _run(x, theta, trace=False)
    return out


def run_traced(x, theta, compute=None):
    """Returns (out, BassKernelResults with exec_time_ns/trace)."""
    return _run(x, theta, trace=True, compute=compute)
